# revision 1
# baseline (speedup 1.0000x reference)
"""Canny NMS kernel V11 for trn2, 8-core data parallel.

Per chunk-group (4 images x 128-row window, merged free dim 2048):
  A = (gray o Gv) band mms (PE fp32, 3 mm/img, half-group PSUM)
  -> Asb (Act) -> blurH p,q (Pool tt) r1,blur (DVE stt, f32)
  -> dxh/shp (Pool) sh (DVE) -> gx = Sv mm, gy = Dv mm (PE fp32,
  half-group PSUM) -> squares f32 + scaled-f16 squares (Act)
  -> msq32, msq2_32 = msq^2 (Pool) -> msq2 f16 (Act)
  -> masks m0/m90 on f16 scaled squares (DVE tt), s45 via pxy
  -> u/d row shifts of msq2 via SBUF-SBUF DMA (partition offset)
  -> 4 direction maxes (DVE f16) -> 3 copy_predicated -> z (f16).

Numerics: NMS compares on f16(msq^2) (squaring halves relative tie
width); blur chain f32 (cancellation); masks on f16 squares.

PSUM: 8 banks as 4 x [128,1024] half-group tiles (A-half x2, GX, GY)
so every cross-group PSUM recurrence is short. Software-pipelined
emission B(g-2); M(g-1); F(g).
"""

import numpy as np

H = W = 512
B = 32
N_CORES = 8
IMGS_PER_CORE = B // N_CORES  # 4

GROUPS = [
    # (s, load_r0, load_r1, dest_p0, z_p0, z_p1)
    (0, 0, 128, 0, 0, 124),      # z rows [0, 124)
    (120, 120, 248, 0, 4, 124),  # [124, 244)
    (240, 240, 368, 0, 4, 124),  # [244, 364)
    (360, 360, 488, 0, 4, 124),  # [364, 484)
    (384, 384, 512, 0, 100, 128),  # [484, 512)
]

_GRAY_W = (0.299, 0.587, 0.114)
T1 = float(np.float32(np.tan(np.deg2rad(22.5))))
T2 = float(np.float32(np.tan(np.deg2rad(67.5))))

NI = IMGS_PER_CORE  # 4
F = NI * W          # 2048
HF = F // 2         # 1024 (half-group)
W2, W4 = W + 2, W + 4
FP = NI * W2
FP2 = NI * W4


def _g1n():
    ax = np.arange(-2, 3, dtype=np.float64)
    g = np.exp(-(ax * ax) / 2.0)
    return g / g.sum()


def _band(w, off):
    Bm = np.zeros((128, 128), np.float32)
    idx = np.arange(128)
    for d, wv in enumerate(w):
        kk = idx + d - off
        valid = (kk >= 0) & (kk < 128)
        Bm[kk[valid], idx[valid]] = np.float32(wv)
    return Bm


def _weights32():
    g1 = _g1n()
    return np.stack([
        _band(_GRAY_W[0] * g1, 2),     # 0: ch0 gray+Gv
        _band(_GRAY_W[1] * g1, 2),     # 1: ch1
        _band(_GRAY_W[2] * g1, 2),     # 2: ch2
        _band([1.0, 2.0, 1.0], 1),     # 3: Sv (gx)
        _band([-1.0, 0.0, 1.0], 1),    # 4: Dv (gy)
    ]).astype(np.float32)


def _weights16():
    return np.stack([
        _band([1.0], 1),               # 0: shift up   u[m] = in[m-1]
        _band([1.0], -1),              # 1: shift down d[m] = in[m+1]
    ]).astype(np.float16)


_NC_CACHE = {}


def _build(n_reps):
    import concourse.bacc as bacc
    import concourse.tile as tile
    from concourse import mybir

    f32 = mybir.dt.float32
    f16 = mybir.dt.float16
    u16 = mybir.dt.uint16
    Alu = mybir.AluOpType
    Act = mybir.ActivationFunctionType

    g1 = _g1n()
    a_ov_b = float(np.float32(g1[0] / g1[1]))
    b_ov_c = float(np.float32(g1[1] / g1[2]))

    nc = bacc.Bacc("TRN2", target_bir_lowering=False, debug=False,
                   num_devices=N_CORES)
    x_d = nc.dram_tensor("x", [NI, 3, H, W], f32, kind="ExternalInput").ap()
    w32_d = nc.dram_tensor("w32", [5, 128, 128], f32,
                           kind="ExternalInput").ap()
    w16_d = nc.dram_tensor("w16", [2, 128, 128], f16,
                           kind="ExternalInput").ap()
    z_d = nc.dram_tensor("zeros", [8, 3 * F], f32, kind="ExternalInput").ap()
    y_d = nc.dram_tensor("y", [NI, H, W], f16, kind="ExternalOutput").ap()

    with tile.TileContext(nc) as tc:
        import contextlib
        with contextlib.ExitStack() as ctx:
            wpool = ctx.enter_context(tc.tile_pool(name="w", bufs=1))
            sb = ctx.enter_context(tc.tile_pool(name="sb", bufs=1))
            ps = ctx.enter_context(tc.tile_pool(name="ps", bufs=1,
                                                space="PSUM"))

            wt32 = wpool.tile([128, 5 * 128], f32)
            nc.sync.dma_start(
                wt32[:].rearrange("k (n m) -> k n m", n=5),
                w32_d.rearrange("n k m -> k n m"))

            wt16 = wpool.tile([128, 2 * 128], f16)
            nc.sync.dma_start(
                wt16[:].rearrange("k (n m) -> k n m", n=2),
                w16_d.rearrange("n k m -> k n m"))

            def wsl(n, kmax=128):
                return wt32[0:kmax, n * 128:(n + 1) * 128]

            # --- physical SBUF buffers ---
            fA = [sb.tile([128, FP2], f32, name=f"fA{i}") for i in range(2)]
            fB = [sb.tile([128, FP2], f32, name=f"fB{i}") for i in range(2)]
            fC = [sb.tile([128, FP2], f32, name=f"fC{i}") for i in range(2)]
            fD = [sb.tile([128, FP2], f32, name=f"fD{i}") for i in range(2)]
            fE = [sb.tile([128, FP2], f32, name=f"fE{i}") for i in range(2)]
            xts = [sb.tile([128, 3 * F], f32, name=f"xt{i}")
                   for i in range(2)]
            msq2t = sb.tile([128, FP], f16, name="msq2t")
            usbt = sb.tile([128, FP], f16, name="usbt")
            dsbt = sb.tile([128, FP], f16, name="dsbt")
            sq16s = [sb.tile([128, 3 * F], f16, name=f"sq16_{i}")
                     for i in range(2)]  # [t1^2gx^2|t2^2gx^2|gy^2]
            m0t = sb.tile([128, F], u16)
            m90t = sb.tile([128, F], u16)
            s45t = sb.tile([128, F], u16)
            pxyts = [sb.tile([128, F], f16, name=f"pxyt{i}")
                     for i in range(2)]
            mselt = sb.tile([128, F], f16)
            m45v = sb.tile([128, F], f16)
            m90v = sb.tile([128, F], f16)
            m0v = sb.tile([128, F], f16)

            # --- PSUM: 4 half-group tiles (2 banks each) ---
            psA = [ps.tile([128, HF], f32, name=f"psA{i}")
                   for i in range(2)]
            psX = ps.tile([128, HF], f32, name="psX")
            psY = ps.tile([128, HF], f32, name="psY")
            zrow16 = z_d[0:1, :].bitcast(f16)[:, 0:FP]

            def pv(t, wpad, lo, hi):
                ap = t[:] if not hasattr(t, 'rearrange') else t
                return ap.rearrange("p (i w) -> p i w", i=NI)[:, :, lo:hi]

            def fv(ap):
                return ap.rearrange("p (i w) -> p i w", i=NI)

            # --- hoisted one-time guard zeroing ---
            for i in range(2):
                nc.gpsimd.memset(pv(fD[i], W4, 0, 2), 0.0)
                nc.gpsimd.memset(pv(fD[i], W4, W + 2, W + 4), 0.0)
                blur_i = fE[i][:, 0:FP]
                nc.gpsimd.memset(pv(blur_i, W2, 0, 1), 0.0)
                nc.gpsimd.memset(pv(blur_i, W2, W + 1, W + 2), 0.0)
            for t in (msq2t, usbt, dsbt):
                nc.gpsimd.memset(pv(t, W2, 0, 1), 0.0)
                nc.gpsimd.memset(pv(t, W2, W + 1, W + 2), 0.0)

            import contextlib as _ctl

            def tiles_for(gi):
                ph = gi % 2
                d = {}
                d["xt"] = xts[ph]
                d["msq2"], d["usb"], d["dsb"] = msq2t, usbt, dsbt
                d["sq16"] = sq16s[ph]
                d["pxyt"] = pxyts[ph]
                d["r1t"] = d["sht"] = d["msq2_32"] = fA[ph][:, 0:F]
                d["pt"] = d["dxh"] = d["gysb"] = fB[ph][:, 0:F]
                d["qt"] = d["shp"] = d["msq32"] = fC[ph][:, 0:F]
                d["Asb"] = fD[ph]
                d["sqx"] = xts[ph][:, 0:F]
                d["blur"] = fE[ph][:, 0:FP]
                d["sqy"] = xts[ph][:, F:2 * F]
                s, r0, r1, p0, zp0, zp1 = GROUPS[gi]
                d.update(s=s, r0=r0, r1=r1, p0=p0, zp0=zp0, zp1=zp1)
                d["kmax"] = 128
                return d

            def load(gi):
                d = tiles_for(gi)
                xt = d["xt"]
                for c in range(3):
                    nc.sync.dma_start(
                        xt[d["p0"]:d["p0"] + d["r1"] - d["r0"],
                           c * F:(c + 1) * F].rearrange(
                            "p (i w) -> p i w", i=NI),
                        x_d.rearrange("i c h w -> c h i w")[c,
                                                            d["r0"]:d["r1"]])

            def front(gi):
                d = tiles_for(gi)
                xt = d["xt"]
                r1t, sht = d["r1t"], d["sht"]
                pt, dxh = d["pt"], d["dxh"]
                qt, shp = d["qt"], d["shp"]
                Asb, blur = d["Asb"], d["blur"]
                sqx, sqy, gysb = d["sqx"], d["sqy"], d["gysb"]
                sq16, pxyt = d["sq16"], d["pxyt"]
                kmax = d["kmax"]
                # ---- A = sum_ch (gray_ch * Gv) @ x_ch (PE fp32, halves)
                for h in range(2):
                    for i2 in range(2):
                        i = 2 * h + i2
                        for c in range(3):
                            nc.tensor.matmul(
                                psA[h][:, i2 * W:(i2 + 1) * W], wsl(c, kmax),
                                xt[0:kmax,
                                   c * F + i * W:c * F + (i + 1) * W],
                                start=(c == 0), stop=(c == 2))
                    # Asb half (Act copy, padded)
                    nc.scalar.copy(
                        pv(Asb, W4, 2, W + 2)[:, 2 * h:2 * h + 2, :],
                        psA[h][:].rearrange("p (i w) -> p i w", i=2))

                # ---- blurH
                nc.gpsimd.tensor_tensor(
                    fv(pt), pv(Asb, W4, 0, W), pv(Asb, W4, 4, W + 4),
                    op=Alu.add)
                nc.vector.tensor_tensor(
                    fv(qt), pv(Asb, W4, 1, W + 1), pv(Asb, W4, 3, W + 3),
                    op=Alu.add)
                nc.vector.scalar_tensor_tensor(
                    r1t, pt, a_ov_b, qt, op0=Alu.mult, op1=Alu.add)
                nc.vector.scalar_tensor_tensor(
                    pv(blur, W2, 1, W + 1), fv(r1t), b_ov_c,
                    pv(Asb, W4, 2, W + 2), op0=Alu.mult, op1=Alu.add)
                # ---- dxh (Pool), shp (Pool), sh (DVE stt)
                nc.gpsimd.tensor_tensor(
                    fv(dxh), pv(blur, W2, 2, W + 2), pv(blur, W2, 0, W),
                    op=Alu.subtract)
                nc.gpsimd.tensor_tensor(
                    fv(shp), pv(blur, W2, 0, W), pv(blur, W2, 2, W + 2),
                    op=Alu.add)
                nc.vector.scalar_tensor_tensor(
                    fv(sht), pv(blur, W2, 1, W + 1), 2.0, fv(shp),
                    op0=Alu.mult, op1=Alu.add)

                # ---- gx -> psX[h], gy -> psY (halves); consume promptly
                for h in range(2):
                    hs = slice(h * HF, (h + 1) * HF)
                    for i2 in range(2):
                        i = 2 * h + i2
                        nc.tensor.matmul(
                            psX[:, i2 * W:(i2 + 1) * W], wsl(3, kmax),
                            dxh[0:kmax, i * W:(i + 1) * W],
                            start=True, stop=True)
                        nc.tensor.matmul(
                            psY[:, i2 * W:(i2 + 1) * W], wsl(4, kmax),
                            sht[0:kmax, i * W:(i + 1) * W],
                            start=True, stop=True)
                    # f32 squares (for msq), f16 scaled squares (for masks)
                    nc.scalar.square(sqx[:, hs], psX[:])
                    nc.scalar.square(sqy[:, hs], psY[:])
                    nc.scalar.activation(
                        sq16[:, 0 * F + h * HF:0 * F + (h + 1) * HF],
                        psX[:], Act.Square, scale=T1)
                    nc.scalar.activation(
                        sq16[:, 1 * F + h * HF:1 * F + (h + 1) * HF],
                        psX[:], Act.Square, scale=T2)
                    nc.scalar.activation(
                        sq16[:, 2 * F + h * HF:2 * F + (h + 1) * HF],
                        psY[:], Act.Square)
                    nc.scalar.copy(gysb[:, hs], psY[:])
                    nc.vector.tensor_tensor(pxyt[:, hs], psX[:],
                                            gysb[:, hs], op=Alu.mult)

            def mid(gi):
                d = tiles_for(gi)
                sqx, sqy, sq16 = d["sqx"], d["sqy"], d["sq16"]
                pxyt = d["pxyt"]
                msq32, msq2_32 = d["msq32"], d["msq2_32"]
                msq2, usb, dsb = d["msq2"], d["usb"], d["dsb"]
                # ---- masks on f16 scaled squares (DVE tt), s45
                nc.vector.tensor_tensor(
                    m0t[:], sq16[:, 0:F], sq16[:, 2 * F:3 * F],
                    op=Alu.is_gt)
                nc.vector.tensor_tensor(
                    m90t[:], sq16[:, F:2 * F], sq16[:, 2 * F:3 * F],
                    op=Alu.is_le)
                nc.vector.tensor_scalar(s45t[:], pxyt[:], 0.0, None,
                                        op0=Alu.is_ge)
                # ---- msq32 (Pool), msq2 = Square(msq32) -> f16 (Act)
                nc.gpsimd.tensor_tensor(msq32, sqx, sqy, op=Alu.add)
                nc.scalar.activation(pv(msq2, W2, 1, W + 1), fv(msq32),
                                     Act.Square)
                # ---- u/d row shifts via PE f16 identity-band mms
                for h in range(2):
                    for i2 in range(2):
                        i = 2 * h + i2
                        rhs = msq2[:, i * W2 + 1:i * W2 + 1 + W]
                        nc.tensor.matmul(
                            psX[:, i2 * W:(i2 + 1) * W],
                            wt16[:, 0:128], rhs, start=True, stop=True)
                        nc.tensor.matmul(
                            psY[:, i2 * W:(i2 + 1) * W],
                            wt16[:, 128:256], rhs, start=True, stop=True)
                    nc.scalar.copy(
                        pv(usb, W2, 1, W + 1)[:, 2 * h:2 * h + 2, :],
                        psX[:].rearrange("p (i w) -> p i w", i=2))
                    nc.scalar.copy(
                        pv(dsb, W2, 1, W + 1)[:, 2 * h:2 * h + 2, :],
                        psY[:].rearrange("p (i w) -> p i w", i=2))


            def back(gi):
                d = tiles_for(gi)
                msq2, usb, dsb = d["msq2"], d["usb"], d["dsb"]
                nc.vector.tensor_tensor(
                    fv(mselt[:]), pv(usb, W2, 0, W),
                    pv(dsb, W2, 2, W + 2), op=Alu.max)
                nc.vector.tensor_tensor(
                    fv(m45v[:]), pv(dsb, W2, 0, W),
                    pv(usb, W2, 2, W + 2), op=Alu.max)
                nc.vector.tensor_tensor(
                    fv(m90v[:]), pv(usb, W2, 1, W + 1),
                    pv(dsb, W2, 1, W + 1), op=Alu.max)
                nc.vector.tensor_tensor(
                    fv(m0v[:]), pv(msq2, W2, 0, W),
                    pv(msq2, W2, 2, W + 2), op=Alu.max)
                nc.vector.copy_predicated(mselt[:], s45t[:], m45v[:])
                nc.vector.copy_predicated(mselt[:], m90t[:], m90v[:])
                nc.vector.copy_predicated(mselt[:], m0t[:], m0v[:])
                # z written into m45v (dead after first pred)
                nc.vector.tensor_tensor(
                    fv(m45v[:]), pv(msq2, W2, 1, W + 1), fv(mselt[:]),
                    op=Alu.is_ge)
                zr0, zr1 = d["s"] + d["zp0"], d["s"] + d["zp1"]
                for i in range(NI):
                    nc.sync.dma_start(
                        y_d[i, zr0:zr1, :],
                        m45v[d["zp0"]:d["zp1"], i * W:(i + 1) * W])

            NG = len(GROUPS)
            rep_ctx = (tc.For_i(0, n_reps, 1) if n_reps > 1
                       else _ctl.nullcontext())
            with rep_ctx:
                # software-pipelined emission:
                # B(g-2); F(g); M(g-1); load(g+1)
                load(0)
                for g in range(NG + 2):
                    if g >= 2:
                        back(g - 2)
                    if g < NG:
                        front(g)
                    if 1 <= g < NG + 1:
                        mid(g - 1)
                    if g + 1 < NG:
                        load(g + 1)
    nc.compile()
    return nc


def _get_nc(n_reps):
    if n_reps not in _NC_CACHE:
        _NC_CACHE[n_reps] = _build(n_reps)
    return _NC_CACHE[n_reps]


def run_on_cores(x, n_reps=1):
    from concourse.bass_utils import run_bass_kernel_spmd

    nc = _get_nc(n_reps)
    w32 = _weights32()
    w16 = _weights16()
    zeros = np.zeros((8, 3 * F), np.float32)
    x = np.ascontiguousarray(np.asarray(x), dtype=np.float32)
    in_maps = [
        {"x": x[c * IMGS_PER_CORE:(c + 1) * IMGS_PER_CORE],
         "w32": w32, "w16": w16, "zeros": zeros}
        for c in range(N_CORES)
    ]
    res = run_bass_kernel_spmd(nc, in_maps, list(range(N_CORES)))
    out = np.concatenate(
        [np.asarray(res.results[c]["y"], dtype=np.float32)[:, None]
         for c in range(N_CORES)], axis=0)
    return out


def kernel(x):
    return run_on_cores(x, n_reps=1)

